# revision 2
# baseline (speedup 1.0000x reference)
"""TRN2 Bass kernel for nn_CustomLinear_66005057405513.

Computes y = FFT_4096(w * x)[:, :3072] for x: [4096, 4096] complex64
(given as interleaved float pairs) and w: [4096] complex64 twiddles.

Strategy: data-parallel over 8 NeuronCores (512 batch rows each). On each
core, a two-step radix-64 FFT with all twiddles folded into precomputed
matrices:

  n = 64*o + i, k = p + 64*q, q < 48:
    stage 1 (per i):  A[b, i, p] = sum_o C1[i][o, p] * x[b, 64o+i]
    stage 2 (per p):  y[b, p+64q] = sum_i C2[p][i, q] * A[b, i, p]

v2 layout (vs v1): x is host-pre-transposed to [(o,c), b] per i so stage-1
is pure matmuls (data tile stationary via LDWEIGHTS, pairmat streamed,
out [b, (p,c)]) with NO PE transposes. The inter-stage corner turn
([b,(i,c)] -> [(i,c),b] tiles, per p) is split between DMA XBAR
transposes and PE transposes (tunable). Stage 2 runs wide: pairmat(C2[p])
is the stationary lhsT, the transposed data tile streams N=512 batch
columns, producing y transposed [(q,c), b] which the host un-permutes
for free.
"""

import numpy as np

import concourse.bass as bass
import concourse.mybir as mybir
from concourse import bacc
from concourse.tile import TileContext
from concourse.masks import make_identity
from concourse.bass_utils import run_bass_kernel_spmd

O = I = 64
N_FFT = O * I          # 4096
Q = 48                 # q < 48  <=>  k < 3072
B_TOTAL = 4096
N_CORES = 8
B_LOCAL = B_TOTAL // N_CORES  # 512
SLABS = B_LOCAL // 128         # 4

# corner-turn split: p uses DMA XBAR transpose when (p % XBAR_MOD) < XBAR_THR
XBAR_MOD = 2
XBAR_THR = 1


def _make_tables(w_complex):
    oo = np.arange(O)
    W64 = np.exp(-2j * np.pi * np.outer(oo, oo) / O)
    WN = np.exp(-2j * np.pi * np.outer(np.arange(I), oo) / N_FFT)

    def pairmat(C):
        K, M = C.shape
        G = np.empty((2 * K, 2 * M), np.float64)
        G[0::2, 0::2] = C.real
        G[1::2, 0::2] = -C.imag
        G[0::2, 1::2] = C.imag
        G[1::2, 1::2] = C.real
        return G

    g1 = np.empty((128, I, 128), np.float64)
    for i in range(I):
        C1 = W64 * w_complex[64 * oo + i][:, None]
        g1[:, i, :] = pairmat(C1)
    g2 = np.empty((128, O, 96), np.float64)
    for p in range(O):
        C2 = WN[:, p][:, None] * W64[:, :Q]
        g2[:, p, :] = pairmat(C2)
    return g1, g2


def _build_nc(compute="f16", act_every=2, reps=1):
    f32 = mybir.dt.float32
    cdt = mybir.dt.float16

    nc = bacc.Bacc(None, target_bir_lowering=False, debug=False)
    # x host-transposed: [(o,c)=128, (s, i, b')] per core
    x = nc.declare_dram_parameter("x", [128, SLABS * I * 128], cdt, isOutput=False)
    w1 = nc.declare_dram_parameter("w1", [128, I * 128], cdt, isOutput=False)
    w2 = nc.declare_dram_parameter("w2", [128, O * 96], cdt, isOutput=False)
    # y transposed: rows (p, (q,c)), cols b
    y = nc.declare_dram_parameter("y", [O * 96, B_LOCAL], cdt, isOutput=True)

    cc = [0]

    def copy(out_ap, in_ap):
        cc[0] += 1
        if not act_every or cc[0] % act_every:
            nc.vector.tensor_copy(out_ap, in_ap)
        else:
            nc.scalar.copy(out_ap, in_ap)

    with TileContext(nc) as tc:
        with (
            tc.tile_pool(name="const", bufs=1) as cpool,
            tc.tile_pool(name="xp", bufs=2) as xpool,
            tc.tile_pool(name="ab", bufs=1) as abpool,
            tc.tile_pool(name="t2", bufs=4) as t2pool,
            tc.tile_pool(name="yp", bufs=4) as ypool,
            tc.tile_pool(name="ps1", bufs=4, space="PSUM") as ps1pool,
            tc.tile_pool(name="pst", bufs=2, space="PSUM") as pstpool,
            tc.tile_pool(name="ps2", bufs=2, space="PSUM") as ps2pool,
        ):
            ident = cpool.tile([128, 128], cdt, name="ident")
            make_identity(nc, ident[:])
            w1s = cpool.tile([128, I * 128], cdt, name="w1s")
            nc.scalar.dma_start(out=w1s[:], in_=w1[:])
            w2s = cpool.tile([128, O * 96], cdt, name="w2s")
            nc.scalar.dma_start(out=w2s[:], in_=w2[:])
            w1v = w1s[:].rearrange("k (i n) -> k i n", i=I)
            w2v = w2s[:].rearrange("k (p n) -> k p n", p=O)

            def job(_iv=None):
                Ab = abpool.tile([128, SLABS * O * I * 2], cdt, name="Ab")
                # Ab free layout: (s, p, i, c)
                av = Ab[:].rearrange("b (s p i c) -> b s p i c",
                                     s=SLABS, p=O, c=2)
                for s in range(SLABS):
                    xs = xpool.tile([128, I * 128], cdt, name="xs")
                    for ch in range(2):
                        nc.sync.dma_start(
                            out=xs[:, ch * 4096:(ch + 1) * 4096],
                            in_=x[:, s * 8192 + ch * 4096:
                                  s * 8192 + (ch + 1) * 4096])
                    for g in range(I // 4):
                        ps1 = ps1pool.tile([128, 512], f32, name="ps1")
                        for j in range(4):
                            i = 4 * g + j
                            nc.tensor.matmul(
                                ps1[:, j * 128:(j + 1) * 128],
                                lhsT=xs[:, i * 128:(i + 1) * 128],
                                rhs=w1v[:, i, :], start=True, stop=True)
                        copy(av[:, s, :, 4 * g:4 * g + 4, :],
                             ps1[:].rearrange("b (j p c) -> b p j c",
                                              j=4, c=2))

                for p in range(O):
                    t2 = t2pool.tile([128, 512], cdt, name="t2")
                    if (p % XBAR_MOD) < XBAR_THR:
                        for s in range(SLABS):
                            eng = nc.sync if s % 2 else nc.scalar
                            eng.dma_start(
                                out=t2[:, s * 128:(s + 1) * 128],
                                in_=Ab[:, (s * O + p) * 128:
                                       (s * O + p) * 128 + 128],
                                transpose=True)
                    else:
                        pst = pstpool.tile([128, 512], cdt, name="pst")
                        for s in range(SLABS):
                            nc.tensor.transpose(
                                pst[:, s * 128:(s + 1) * 128],
                                Ab[:, (s * O + p) * 128:
                                   (s * O + p) * 128 + 128],
                                ident[:])
                        copy(t2[:], pst[:])
                    ps2 = ps2pool.tile([96, 512], f32, name="ps2")
                    nc.tensor.matmul(ps2[:], lhsT=w2v[:, p, :], rhs=t2[:],
                                     start=True, stop=True)
                    ys = ypool.tile([96, 512], cdt, name="ys")
                    copy(ys[:], ps2[:])
                    nc.scalar.dma_start(out=y[p * 96:(p + 1) * 96, :],
                                        in_=ys[:])

            if reps > 1:
                with tc.For_i(0, reps, 1) as _i:
                    job(_i)
            else:
                job()

    nc.compile()
    return nc


_NC_CACHE = {}


def _get_nc(compute="f16"):
    if compute not in _NC_CACHE:
        _NC_CACHE[compute] = _build_nc(compute)
    return _NC_CACHE[compute]


def _host_inputs(x_real, weights_real, compute="f16"):
    np_dt = np.float16
    wr = np.asarray(weights_real, dtype=np.float64)
    wc = wr[0::2] + 1j * wr[1::2]
    g1, g2 = _make_tables(wc)
    w1 = np.ascontiguousarray(g1.reshape(128, -1)).astype(np_dt)
    w2 = np.ascontiguousarray(g2.reshape(128, -1)).astype(np_dt)
    x = np.asarray(x_real)
    B = x.shape[0]
    # [core, s, b', o, i, c] -> [core, o, c, s, i, b']
    xf = x.reshape(N_CORES, SLABS, 128, O, I, 2).transpose(0, 3, 5, 1, 4, 2)
    xf = np.ascontiguousarray(xf).reshape(N_CORES, 128, -1).astype(np_dt)
    return [{"x": xf[c], "w1": w1, "w2": w2} for c in range(N_CORES)]


def kernel(x_real, weights_real):
    nc = _get_nc()
    in_maps = _host_inputs(x_real, weights_real)
    res = run_bass_kernel_spmd(nc, in_maps, list(range(N_CORES)))
    outs = []
    for c in range(N_CORES):
        v = np.asarray(res.results[c]["y"], dtype=np.float32)
        # [p, q, c, b] -> complex [b, q, p] -> [b, 64q + p]
        v = v.reshape(O, Q, 2, B_LOCAL)
        z = (v[:, :, 0, :] + 1j * v[:, :, 1, :]).astype(np.complex64)
        outs.append(np.ascontiguousarray(z.transpose(2, 1, 0)).reshape(
            B_LOCAL, Q * O))
    return np.concatenate(outs, axis=0)


# revision 10
# speedup vs baseline: 2.5503x; 2.5503x over previous
"""TRN2 Bass kernel for nn_CustomLinear_66005057405513.

Computes y = FFT_4096(w * x)[:, :3072] for x: [4096, 4096] complex64
(given as interleaved float pairs) and w: [4096] complex64 twiddles.

Strategy: data-parallel over 8 NeuronCores (512 batch rows each). On each
core, a two-step radix-64 FFT with all twiddles folded into precomputed
matrices:

  n = 64*o + i, k = p + 64*q, q < 48:
    stage 1 (per i):  A[b, i, p] = sum_o C1[i][o, p] * x[b, 64o+i]
    stage 2 (per p):  y[b, p+64q] = sum_i C2[p][i, q] * A[b, i, p]

v4: x host-pre-transposed to [(o,c), b] per i so stage-1 is pure matmuls
(x tile stationary, pairmat streamed, out [b, (p,c)]) with no PE
transposes. Corner turn [b,(i,c)] -> [(i,c),b] on the PE, software-
pipelined (in pairs of p) ahead of the wide stage-2 matmuls
(pairmat(C2[p]) stationary, N=512). y leaves transposed [(q,c), b]; the
host un-permutes for free. PSUM groups are 1024 wide (2 banks) so each
PSUM->SBUF copy moves 1024 columns; copies rotate over
vector/scalar/gpsimd weighted by engine speed.
"""

import numpy as np

import concourse.bass as bass
import concourse.mybir as mybir
from concourse import bacc
from concourse.tile import TileContext
from concourse.masks import make_identity
from concourse.bass_utils import run_bass_kernel_spmd

O = I = 64
N_FFT = O * I
Q = 48
B_TOTAL = 4096
N_CORES = 8
B_LOCAL = B_TOTAL // N_CORES  # 512
SLABS = B_LOCAL // 128         # 4

DEPTH = 2          # p-pair pipeline depth (corner turn leads stage-2)
COPY_ENGS = "vs"   # copy-engine rotation (gpsimd cannot read PSUM)


def _make_tables(w_complex):
    oo = np.arange(O)
    W64 = np.exp(-2j * np.pi * np.outer(oo, oo) / O)
    WN = np.exp(-2j * np.pi * np.outer(np.arange(I), oo) / N_FFT)

    def pairmat(C):
        K, M = C.shape
        G = np.empty((2 * K, 2 * M), np.float64)
        G[0::2, 0::2] = C.real
        G[1::2, 0::2] = -C.imag
        G[0::2, 1::2] = C.imag
        G[1::2, 1::2] = C.real
        return G

    g1 = np.empty((128, I, 128), np.float64)
    for i in range(I):
        C1 = W64 * w_complex[64 * oo + i][:, None]
        g1[:, i, :] = pairmat(C1)
    g2 = np.empty((128, O, 96), np.float64)
    for p in range(O):
        C2 = WN[:, p][:, None] * W64[:, :Q]
        g2[:, p, :] = pairmat(C2)
    return g1, g2


def _build_nc(compute="f16", act_every=2, reps=1, unroll=False):
    f32 = mybir.dt.float32
    cdt = mybir.dt.float16

    nc = bacc.Bacc(None, target_bir_lowering=False, debug=False)
    x = nc.declare_dram_parameter("x", [128, SLABS * I * 128], cdt, isOutput=False)
    w1 = nc.declare_dram_parameter("w1", [128, I * 128], cdt, isOutput=False)
    w2 = nc.declare_dram_parameter("w2", [128, O * 96], cdt, isOutput=False)
    y = nc.declare_dram_parameter("y", [O * 96, B_LOCAL], cdt, isOutput=True)

    cc = [0]
    engs = [{"v": nc.vector, "s": nc.scalar, "g": nc.gpsimd}[ch]
            for ch in COPY_ENGS]

    def copy(out_ap, in_ap):
        e = engs[cc[0] % len(engs)]
        cc[0] += 1
        if e is nc.scalar:
            e.copy(out_ap, in_ap)
        else:
            e.tensor_copy(out_ap, in_ap)

    with TileContext(nc) as tc:
        with (
            tc.tile_pool(name="const", bufs=1) as cpool,
            tc.tile_pool(name="xp", bufs=2) as xpool,
            tc.tile_pool(name="ab", bufs=2) as abpool,
            tc.tile_pool(name="t2", bufs=DEPTH + 1) as t2pool,
            tc.tile_pool(name="yp", bufs=3) as ypool,
            tc.tile_pool(name="psA", bufs=3, space="PSUM") as psA,
            tc.tile_pool(name="psT", bufs=2, space="PSUM") as psT,
        ):
            ident = cpool.tile([128, 128], cdt, name="ident")
            make_identity(nc, ident[:])
            w1s = cpool.tile([128, I * 128], cdt, name="w1s")
            nc.scalar.dma_start(out=w1s[:], in_=w1[:])
            w2s = cpool.tile([128, O * 96], cdt, name="w2s")
            nc.scalar.dma_start(out=w2s[:], in_=w2[:])
            w1v = w1s[:].rearrange("k (i n) -> k i n", i=I)
            w2v = w2s[:].rearrange("k (p n) -> k p n", p=O)

            def job(_iv=None):
                Ab = abpool.tile([128, SLABS * O * I * 2], cdt, name="Ab")
                av = Ab[:].rearrange("b (s p i c) -> b s p i c",
                                     s=SLABS, p=O, c=2)
                for s in range(SLABS):
                    for hf in range(2):
                        # half-slab x tile: i in [32*hf, 32*hf+32)
                        xs = xpool.tile([128, I * 64], cdt, name="xs")
                        nc.sync.dma_start(
                            out=xs[:],
                            in_=x[:, s * 8192 + hf * 4096:
                                  s * 8192 + (hf + 1) * 4096])
                        for g in range(4):
                            ps1 = psA.tile([128, 1024], f32, name="ps1")
                            for j in range(8):
                                i = 32 * hf + 8 * g + j
                                nc.tensor.matmul(
                                    ps1[:, j * 128:(j + 1) * 128],
                                    lhsT=xs[:, (i % 32) * 128:
                                            (i % 32 + 1) * 128],
                                    rhs=w1v[:, i, :], start=True, stop=True)
                            i0 = 32 * hf + 8 * g
                            copy(av[:, s, :, i0:i0 + 8, :],
                                 ps1[:].rearrange("b (j p c) -> b p j c",
                                                  j=8, c=2))

                t2s = {}
                yv = y[:].rearrange("(p r) b -> r p b", r=96)

                def turn(u):
                    pst = psT.tile([128, 1024], cdt, name="pst")
                    for h in range(2):
                        p = 2 * u + h
                        for s in range(SLABS):
                            nc.tensor.transpose(
                                pst[:, h * 512 + s * 128:
                                    h * 512 + (s + 1) * 128],
                                Ab[:, (s * O + p) * 128:
                                   (s * O + p) * 128 + 128],
                                ident[:])
                    t2 = t2pool.tile([128, 1024], cdt, name="t2")
                    copy(t2[:], pst[:])
                    t2s[u] = t2

                def fft2(u):
                    ps2 = psA.tile([128, 1024], f32, name="ps1")
                    t2 = t2s.pop(u)
                    for h in range(2):
                        nc.tensor.matmul(
                            ps2[0:96, h * 512:(h + 1) * 512],
                            lhsT=w2v[:, 2 * u + h, :],
                            rhs=t2[:, h * 512:(h + 1) * 512],
                            start=True, stop=True)
                    ys = ypool.tile([96, 1024], cdt, name="ys")
                    copy(ys[:], ps2[0:96, :])
                    nc.scalar.dma_start(
                        out=yv[:, 2 * u:2 * u + 2, :],
                        in_=ys[:].rearrange("r (h b) -> r h b", h=2))

                for uu in range(O // 2 + DEPTH):
                    if uu < O // 2:
                        turn(uu)
                    if uu >= DEPTH:
                        fft2(uu - DEPTH)

            if reps > 1 and unroll:
                for _ in range(reps):
                    job()
            elif reps > 1:
                with tc.For_i(0, reps, 1) as _i:
                    job(_i)
            else:
                job()

    nc.compile()
    return nc


_NC_CACHE = {}


def _get_nc(compute="f16"):
    if compute not in _NC_CACHE:
        _NC_CACHE[compute] = _build_nc(compute)
    return _NC_CACHE[compute]


def _host_inputs(x_real, weights_real, compute="f16"):
    np_dt = np.float16
    wr = np.asarray(weights_real, dtype=np.float64)
    wc = wr[0::2] + 1j * wr[1::2]
    g1, g2 = _make_tables(wc)
    w1 = np.ascontiguousarray(g1.reshape(128, -1)).astype(np_dt)
    w2 = np.ascontiguousarray(g2.reshape(128, -1)).astype(np_dt)
    x = np.asarray(x_real)
    # [core, s, b', o, i, c] -> [core, o, c, s, i, b']
    xf = x.reshape(N_CORES, SLABS, 128, O, I, 2).transpose(0, 3, 5, 1, 4, 2)
    xf = np.ascontiguousarray(xf).reshape(N_CORES, 128, -1).astype(np_dt)
    return [{"x": xf[c], "w1": w1, "w2": w2} for c in range(N_CORES)]


def kernel(x_real, weights_real):
    nc = _get_nc()
    in_maps = _host_inputs(x_real, weights_real)
    res = run_bass_kernel_spmd(nc, in_maps, list(range(N_CORES)))
    outs = []
    for c in range(N_CORES):
        v = np.asarray(res.results[c]["y"], dtype=np.float32)
        v = v.reshape(O, Q, 2, B_LOCAL)
        z = (v[:, :, 0, :] + 1j * v[:, :, 1, :]).astype(np.complex64)
        outs.append(np.ascontiguousarray(z.transpose(2, 1, 0)).reshape(
            B_LOCAL, Q * O))
    return np.concatenate(outs, axis=0)
